# revision 29
# baseline (speedup 1.0000x reference)
"""Tensor-parallel attention kernel for 8 Trainium2 NeuronCores (v2, fp16).

Shards the 32 attention heads across 8 cores (4 heads each): wq/wk/wv are
sharded column-wise, wo row-wise; x is replicated. Each core computes a
partial output (its heads' contribution through wo) and the host sums the
8 partials.

v2 changes vs v1 (f32r baseline, 1.75ms):
- fp16 storage end-to-end (same PE rate as f32r at >=256 moving, but half
  the DMA/SBUF and full rate at narrow widths).
- Single projection pass computing Q,K,V together (x streamed once; 3
  matmuls per stationary load so LDWEIGHTS hides).
- V stays SBUF-resident (no DRAM round-trip); only rope'd Q/K round-trip.
- Host pre-tiles x/w/cos/mask into SBUF layout (contiguous per-partition
  DMA lines).
- Attention: diagonal blocks narrowed to their unmasked column range,
  softmax pipelined with lookahead, denominators via fast approx
  reciprocal off the critical path.
"""

import math
import sys

sys.path.insert(0, "/opt/trn_rl_repo")

import numpy as np

import concourse.bacc as bacc
import concourse.bass as bass
import concourse.mybir as mybir
import concourse.tile as tile
from bass_rust import add_dep_helper

F32 = mybir.dt.float32
F16 = mybir.dt.float16
AF = mybir.ActivationFunctionType
ALU = mybir.AluOpType

HEAD_DIM = 128
NEG = -60000.0


class Cfg:
    def __init__(self, B=2, S=2048, D=4096, H_PER=4, n_cores=8):
        self.B, self.S, self.D, self.H_PER = B, S, D, H_PER
        self.n_cores = n_cores
        self.T = B * S                    # total tokens (batch-major)
        self.O = H_PER * HEAD_DIM         # per-core projection width
        self.DC = D // 128                # contraction chunks
        self.TT = self.T // 128           # 128-token tiles
        self.TM = self.T // 256           # 256-token macro tiles (proj phase)
        self.NJT = S // 128               # max j-tiles per batch
        self.IMACS = S // 512             # 512-wide i-macros per batch
        self.EC = D // 512                # 512-wide e-chunks of the output


def build(cfg: Cfg) -> bacc.Bacc:
    B, S, D, T, O = cfg.B, cfg.S, cfg.D, cfg.T, cfg.O
    H_PER, DC, TT, TM = cfg.H_PER, cfg.DC, cfg.TT, cfg.TM
    NJT, EC = cfg.NJT, cfg.EC
    scale = 1.0 / math.sqrt(HEAD_DIM)

    nc = bacc.Bacc(None, target_bir_lowering=False)

    # register -7.0 as a usable activation-bias constant (exp logit shift)
    EBIAS = -7.0
    _ebias_t = nc.alloc_sbuf_tensor("ebias_const", [128, 1], F32)
    nc.gpsimd.memset(_ebias_t.ap(), EBIAS)
    nc.const_aps.aps[(F32, EBIAS)] = _ebias_t.ap()

    # host-pre-tiled inputs (SBUF layout: partition-major)
    xtt = nc.dram_tensor("xtt", [TM, 128, DC, 256], F16, kind="ExternalInput")
    wqt = nc.dram_tensor("wqt", [128, DC, O], F16, kind="ExternalInput")
    wkt = nc.dram_tensor("wkt", [128, DC, O], F16, kind="ExternalInput")
    wvt = nc.dram_tensor("wvt", [128, DC, O], F16, kind="ExternalInput")
    wot = nc.dram_tensor("wot", [128, H_PER, D], F16, kind="ExternalInput")
    cost = nc.dram_tensor("cost", [128, TT, 64], F16, kind="ExternalInput")
    sint = nc.dram_tensor("sint", [128, TT, 64], F16, kind="ExternalInput")
    maskt = nc.dram_tensor("maskt", [128, 4, 512], F16, kind="ExternalInput")
    ident = nc.dram_tensor("ident", [128, 128], F16, kind="ExternalInput")
    onesm = nc.dram_tensor("onesm", [128, 128], F16, kind="ExternalInput")
    out = nc.dram_tensor("out", [T, D], F16, kind="ExternalOutput")

    # DRAM scratch for rope'd q/k in [token, O] layout
    q_s = nc.dram_tensor("q_s", [T, O], F16)
    k_s = nc.dram_tensor("k_s", [T, O], F16)

    with tile.TileContext(nc) as tc:
        with tc.tile_pool(name="const", bufs=1) as cpool, \
             tc.tile_pool(name="vres", bufs=1) as vpool:
            mask_sb = cpool.tile([128, 4, 512], F16, name="mask_sb")
            id_sb = cpool.tile([128, 128], F16, name="id_sb")
            ones_sb = cpool.tile([128, 128], F16, name="ones_sb")
            cos_sb = cpool.tile([128, TT, 64], F16, name="cos_sb")
            sin_sb = cpool.tile([128, TT, 64], F16, name="sin_sb")

            # V for all heads, [tok-in-tile, tile, O] — SBUF-resident
            v_all = vpool.tile([128, TT, O], F16, name="v_all")
            # ld tiles live outside the phase pools so attention's q/k loads
            # can prefetch during the projection tail
            ld_tiles = {w: [vpool.tile([128, 4, 128], F16, name=f"ld{w}{i}",
                                       tag=f"ld{w}{i}")
                            for i in range(4)] for w in ("q", "k")}

            # ============ Phase 1: Q,K,V projections + RoPE ============
            with tc.tile_pool(name="wqkv", bufs=1) as wpool, \
                 tc.tile_pool(name="p1sb", bufs=1) as sb, \
                 tc.tile_pool(name="p1ps", bufs=1, space="PSUM") as ps:
                wq_sb = wpool.tile([128, DC, O], F16, name="wq_sb")
                wk_sb = wpool.tile([128, DC, O], F16, name="wk_sb")
                wv_sb = wpool.tile([128, DC, O], F16, name="wv_sb")
                wtrip = ((wq_sb, wqt), (wk_sb, wkt), (wv_sb, wvt))

                def w_chunk(c0, cn):
                    for w_sb, wdram in wtrip:
                        nc.sync.dma_start(out=w_sb[:, c0:c0 + cn, :],
                                          in_=wdram[:, c0:c0 + cn, :])

                # single DMA queue is FIFO: order strictly by first need.
                # tm=0's x pieces are emitted here, interleaved with weights.
                x_sb0 = sb.tile([128, DC, 256], F16, name="x_sb", tag="x_sb", bufs=2)
                w_chunk(0, 2)
                nc.sync.dma_start(out=x_sb0[:, 0:8, :], in_=xtt[0][:, 0:8, :])
                w_chunk(2, 6)
                nc.sync.dma_start(out=x_sb0[:, 8:16, :], in_=xtt[0][:, 8:16, :])
                nc.sync.dma_start(out=mask_sb[:], in_=maskt[:])
                nc.sync.dma_start(out=id_sb[:], in_=ident[:])
                nc.sync.dma_start(out=ones_sb[:], in_=onesm[:])
                nc.sync.dma_start(out=cos_sb[:], in_=cost[:])
                nc.sync.dma_start(out=sin_sb[:], in_=sint[:])
                w_chunk(8, 8)
                nc.sync.dma_start(out=x_sb0[:, 16:32, :], in_=xtt[0][:, 16:32, :])
                w_chunk(16, 8)
                w_chunk(24, 8)

                rot_dmas = {"q": {}, "k": {}}  # tt_i -> rot write DMA inst

                def rope_unit(cp, tt_i, dst):
                    """cp: [128,512] f32 (copied psum); writes rot fp16 + DMA."""
                    rot = sb.tile([128, 512], F16, name="rot", tag="rot", bufs=3)
                    cosb = cos_sb[:, tt_i, :].unsqueeze(1).broadcast_to([128, H_PER, 64])
                    sinb = sin_sb[:, tt_i, :].unsqueeze(1).broadcast_to([128, H_PER, 64])
                    pv = cp[:].rearrange("p (h e k) -> p h e k", e=2, k=64)
                    rv = rot[:].rearrange("p (h e k) -> p h e k", e=2, k=64)
                    pe, po = pv[:, :, 0, :], pv[:, :, 1, :]
                    re, ro = rv[:, :, 0, :], rv[:, :, 1, :]
                    tmp = sb.tile([128, H_PER, 64], F32, name="tmp", tag="tmp", bufs=2)
                    tmp2 = sb.tile([128, H_PER, 64], F32, name="tmp2", tag="tmp2", bufs=2)
                    tmp3 = sb.tile([128, H_PER, 64], F32, name="tmp3", tag="tmp3", bufs=2)
                    nc.gpsimd.tensor_tensor(tmp[:], po, sinb, ALU.mult)
                    nc.gpsimd.tensor_tensor(tmp2[:], po, cosb, ALU.mult)
                    nc.gpsimd.tensor_tensor(tmp3[:], pe, sinb, ALU.mult)
                    nc.vector.tensor_tensor(re, pe, cosb, ALU.mult)
                    nc.vector.tensor_tensor(re, re, tmp[:], ALU.subtract)
                    nc.vector.tensor_tensor(ro, tmp2[:], tmp3[:], ALU.add)
                    dma = nc.sync.dma_start(
                        out=dst[tt_i * 128:(tt_i + 1) * 128, :], in_=rot[:])
                    rot_dmas["q" if dst is q_s else "k"][tt_i] = dma.ins

                for tm in range(TM):
                    if tm == 0:
                        x_sb = x_sb0
                    else:
                        x_sb = sb.tile([128, DC, 256], F16, name="x_sb",
                                       tag="x_sb", bufs=2)
                        for pc in range(2):
                            c0, c1 = DC * pc // 2, DC * (pc + 1) // 2
                            nc.sync.dma_start(out=x_sb[:, c0:c1, :],
                                              in_=xtt[tm][:, c0:c1, :])
                    pp = {}
                    for w in range(3):
                        for ts in range(2):
                            pp[w, ts] = ps.tile([128, 512], F32, name=f"pp{w}{ts}",
                                                tag=f"pp{w}{ts}", bufs=1)
                    for d in range(DC):
                        for ts in range(2):
                            lhs = x_sb[:, d, ts * 128:(ts + 1) * 128]
                            for w, w_sb in enumerate((wq_sb, wk_sb, wv_sb)):
                                nc.tensor.matmul(pp[w, ts][:], lhs, w_sb[:, d, :],
                                                 start=(d == 0), stop=(d == DC - 1))
                    # free psum banks first (in next-needed order), then rope math
                    cps = {}
                    for ts in range(2):
                        for w in range(2):
                            cps[w, ts] = sb.tile([128, 512], F32, name="cp",
                                                 tag=f"cp{w}{ts}", bufs=2)
                            nc.vector.tensor_copy(cps[w, ts][:], pp[w, ts][:])
                        nc.vector.tensor_copy(v_all[:, tm * 2 + ts, :], pp[2, ts][:])
                    for ts in range(2):
                        rope_unit(cps[0, ts], tm * 2 + ts, q_s)
                        rope_unit(cps[1, ts], tm * 2 + ts, k_s)

            # ================= Phase 2: attention per head ==============
            with tc.tile_pool(name="wo", bufs=1) as wopool, \
                 tc.tile_pool(name="otp", bufs=1) as otpool:
              wo_sb = wopool.tile([128, H_PER, D], F16, name="wo_sb")
              for hh in range(H_PER):
                  nc.sync.dma_start(out=wo_sb[:, hh, :], in_=wot[:, hh, :])
              ot_sb = [otpool.tile([128, T], F16, name=f"ot{h}") for h in range(H_PER)]

              with tc.tile_pool(name="atsb", bufs=1) as sb, \
                   tc.tile_pool(name="atps", bufs=1, space="PSUM") as ps:
                for h in range(H_PER):
                    qt_h = sb.tile([128, T], F16, name="qt_h", tag="qt_h", bufs=2)
                    kt_h = sb.tile([128, T], F16, name="kt_h", tag="kt_h", bufs=2)
                    for tp in range(TT // 4):
                        for which, src, dstt in (("q", q_s, qt_h), ("k", k_s, kt_h)):
                            ld = ld_tiles[which][tp % 4]
                            r0 = tp * 512
                            ld_dma = nc.sync.dma_start(
                                out=ld[:],
                                in_=src[r0:r0 + 512, h * 128:(h + 1) * 128]
                                .rearrange("(g p) e -> p g e", p=128))
                            for g in range(4):
                                add_dep_helper(ld_dma.ins,
                                               rot_dmas[which][tp * 4 + g],
                                               True, "q_s/k_s DRAM RAW")
                            for g in range(4):
                                ptr = ps.tile([128, 128], F16, name="ptr",
                                              tag="p_st", bufs=4)
                                nc.tensor.transpose(ptr[:], ld[:, g, :], id_sb[:])
                                nc.vector.tensor_copy(
                                    dstt[:, r0 + g * 128:r0 + (g + 1) * 128], ptr[:])

                    pt_st = sb.tile([128, NJT, 512], F16, name="pt_st",
                                    tag="pt_st", bufs=2)
                    for b in range(B):
                        for im in range(cfg.IMACS):
                            i0 = b * S + im * 512
                            njt = 4 * (im + 1)
                            # diag tiles first; k-th diag tile only covers
                            # columns >= 128k (rest is fully masked)
                            info = []
                            for k in range(4):
                                info.append((4 * im + k, 128 * k, True))
                            for jt in range(4 * im):
                                info.append((jt, 0, False))
                            p_ot = ps.tile([128, 512], F32, name="p_ot",
                                           tag="p_ot", bufs=2)
                            p_r = ps.tile([128, 512], F32, name="p_r",
                                          tag="p_r", bufs=2)

                            def consume(oi):
                                # mixed-extent accumulation group: the k=0
                                # diag tile (full width) opens it; later
                                # narrowed tiles accumulate sub-ranges. stop
                                # may land on a partial range (sim metadata
                                # only), hence skip_group_check.
                                jt, ist, _ = info[oi]
                                nc.tensor.matmul(
                                    p_r[:, ist:], ones_sb[:],
                                    pt_st[:, jt, ist:],
                                    start=(oi == 0), stop=(oi == njt - 1),
                                    skip_group_check=True)
                                nc.tensor.matmul(
                                    p_ot[:, ist:],
                                    v_all[:, b * NJT + jt, h * 128:(h + 1) * 128],
                                    pt_st[:, jt, ist:],
                                    start=(oi == 0), stop=(oi == njt - 1),
                                    skip_group_check=True)

                            LA = 2
                            for oi, (jt, ist, diag) in enumerate(info):
                                j0 = b * S + jt * 128
                                p_st = ps.tile([128, 512], F32, name="p_st",
                                               tag="p_st", bufs=4)
                                nc.tensor.matmul(p_st[:, ist:], kt_h[:, j0:j0 + 128],
                                                 qt_h[:, i0 + ist:i0 + 512],
                                                 start=True, stop=True)
                                # bias shifts all logits by a constant, which
                                # cancels in the softmax normalization; keeps
                                # exp() within fp16 range (logits up to ~18)
                                nc.scalar.activation(pt_st[:, jt, ist:],
                                                     p_st[:, ist:], AF.Exp,
                                                     bias=EBIAS, scale=scale)
                                if diag:
                                    # zero the causally-masked wedge (0/1 mask)
                                    nc.vector.tensor_tensor(
                                        pt_st[:, jt, ist:], pt_st[:, jt, ist:],
                                        mask_sb[:, jt - 4 * im, ist:], ALU.mult)
                                if oi >= LA:
                                    consume(oi - LA)
                            for oi in range(max(0, njt - LA), njt):
                                consume(oi)
                            r_sb = sb.tile([128, 512], F32, name="r_sb",
                                           tag="r_sb", bufs=2)
                            nc.vector.reciprocal_approx_fast(out=r_sb[:], in_=p_r[:])
                            nc.vector.tensor_tensor(ot_sb[h][:, i0:i0 + 512],
                                                    p_ot[:], r_sb[:], ALU.mult)

              # ============= Phase 3: output projection ===============
              with tc.tile_pool(name="p3sb", bufs=1) as sb3, \
                   tc.tile_pool(name="p3ps", bufs=1, space="PSUM") as ps3:
                  for tt_i in range(TT):
                      pf = [ps3.tile([128, 512], F32, name=f"pf{e}", tag=f"pf{e}")
                            for e in range(EC)]
                      for h in range(H_PER):
                          lhs = ot_sb[h][:, tt_i * 128:(tt_i + 1) * 128]
                          for e in range(EC):
                              nc.tensor.matmul(
                                  pf[e][:], lhs,
                                  wo_sb[:, h, e * 512:(e + 1) * 512],
                                  start=(h == 0), stop=(h == H_PER - 1))
                      f_sb = sb3.tile([128, D], F16, name="f_sb", tag="f_sb", bufs=3)
                      for e in range(EC):
                          nc.vector.tensor_copy(f_sb[:, e * 512:(e + 1) * 512],
                                                pf[e][:])
                      nc.sync.dma_start(
                          out=out[tt_i * 128:(tt_i + 1) * 128, :], in_=f_sb[:])

    nc.compile()
    return nc


# host-side even/odd permutation of head dims (RoPE becomes half-split elementwise)
_PERM = np.concatenate([np.arange(0, HEAD_DIM, 2), np.arange(1, HEAD_DIM, 2)])


def host_inputs(cfg: Cfg, x, wq, wk, wv, wo, freqs_cos, freqs_sin):
    """Build the 8 per-core input maps from full inputs (numpy f32)."""
    B, S, D, T, O, H_PER = cfg.B, cfg.S, cfg.D, cfg.T, cfg.O, cfg.H_PER
    TM, DC, TT = cfg.TM, cfg.DC, cfg.TT

    x16 = np.asarray(x, np.float32).reshape(T, D).astype(np.float16)
    # [TM, 128, DC, 256]: xtt[tm, p, dc, j] = x[tm*256 + j, dc*128 + p]
    xtt = np.ascontiguousarray(
        x16.reshape(TM, 256, DC, 128).transpose(0, 3, 2, 1))

    def tile_w(w_cols):  # [D, O] -> [128, DC, O]
        return np.ascontiguousarray(
            w_cols.astype(np.float16).reshape(DC, 128, O).transpose(1, 0, 2))

    cos_full = np.tile(np.asarray(freqs_cos, np.float32), (B, 1)).astype(np.float16)
    sin_full = np.tile(np.asarray(freqs_sin, np.float32), (B, 1)).astype(np.float16)
    cost = np.ascontiguousarray(cos_full.reshape(TT, 128, 64).transpose(1, 0, 2))
    sint = np.ascontiguousarray(sin_full.reshape(TT, 128, 64).transpose(1, 0, 2))

    # maskt[p, k, i] = 1 if 128k + p <= i else 0 (multiplicative causal mask)
    j_idx = (np.arange(4)[None, :, None] * 128 + np.arange(128)[:, None, None])
    i_idx = np.arange(512)[None, None, :]
    maskt = np.where(j_idx <= i_idx, 1.0, 0.0).astype(np.float16)

    ident = np.eye(128, dtype=np.float16)
    onesm = np.ones((128, 128), dtype=np.float16)

    wq = np.asarray(wq, np.float32)
    wk = np.asarray(wk, np.float32)
    wv = np.asarray(wv, np.float32)
    wo = np.asarray(wo, np.float32)

    in_maps = []
    for c in range(cfg.n_cores):
        rows = []
        for hh in range(H_PER):
            base = (c * H_PER + hh) * HEAD_DIM
            rows.append(base + _PERM)
        prows = np.concatenate(rows)                     # permuted rows for q/k
        nrows = np.arange(c * O, (c + 1) * O)            # natural rows for v
        wot_t = np.ascontiguousarray(
            wo[:, nrows].T.astype(np.float16).reshape(H_PER, 128, D)
            .transpose(1, 0, 2))
        in_maps.append({
            "xtt": xtt,
            "wqt": tile_w(wq[prows].T),
            "wkt": tile_w(wk[prows].T),
            "wvt": tile_w(wv[nrows].T),
            "wot": wot_t,
            "cost": cost, "sint": sint, "maskt": maskt,
            "ident": ident, "onesm": onesm,
        })
    return in_maps


_CACHE = {}


def kernel(x, wq, wk, wv, wo, freqs_cos, freqs_sin, mask=None, start_pos=0):
    cfg = Cfg()
    in_maps = host_inputs(cfg, x, wq, wk, wv, wo, freqs_cos, freqs_sin)
    if "nc" not in _CACHE:
        _CACHE["nc"] = build(cfg)
    from concourse.bass_utils import run_bass_kernel_spmd
    res = run_bass_kernel_spmd(_CACHE["nc"], in_maps, core_ids=list(range(cfg.n_cores)))
    acc = np.zeros((cfg.T, cfg.D), dtype=np.float32)
    for c in range(cfg.n_cores):
        acc += res.results[c]["out"].astype(np.float32)
    return acc.reshape(cfg.B, cfg.S, cfg.D)


# revision 33
# speedup vs baseline: 1.0074x; 1.0074x over previous
"""Tensor-parallel attention kernel for 8 Trainium2 NeuronCores (v2, fp16).

Shards the 32 attention heads across 8 cores (4 heads each): wq/wk/wv are
sharded column-wise, wo row-wise; x is replicated. Each core computes a
partial output (its heads' contribution through wo) and the host sums the
8 partials.

v2 changes vs v1 (f32r baseline, 1.75ms):
- fp16 storage end-to-end (same PE rate as f32r at >=256 moving, but half
  the DMA/SBUF and full rate at narrow widths).
- Single projection pass computing Q,K,V together (x streamed once; 3
  matmuls per stationary load so LDWEIGHTS hides).
- V stays SBUF-resident (no DRAM round-trip); only rope'd Q/K round-trip.
- Host pre-tiles x/w/cos/mask into SBUF layout (contiguous per-partition
  DMA lines).
- Attention: diagonal blocks narrowed to their unmasked column range,
  softmax pipelined with lookahead, denominators via fast approx
  reciprocal off the critical path.
"""

import math
import sys

sys.path.insert(0, "/opt/trn_rl_repo")

import numpy as np

import concourse.bacc as bacc
import concourse.bass as bass
import concourse.mybir as mybir
import concourse.tile as tile
from bass_rust import add_dep_helper

F32 = mybir.dt.float32
F16 = mybir.dt.float16
AF = mybir.ActivationFunctionType
ALU = mybir.AluOpType

HEAD_DIM = 128
NEG = -60000.0


class Cfg:
    def __init__(self, B=2, S=2048, D=4096, H_PER=4, n_cores=8):
        self.B, self.S, self.D, self.H_PER = B, S, D, H_PER
        self.n_cores = n_cores
        self.T = B * S                    # total tokens (batch-major)
        self.O = H_PER * HEAD_DIM         # per-core projection width
        self.DC = D // 128                # contraction chunks
        self.TT = self.T // 128           # 128-token tiles
        self.TM = self.T // 256           # 256-token macro tiles (proj phase)
        self.NJT = S // 128               # max j-tiles per batch
        self.IMACS = S // 512             # 512-wide i-macros per batch
        self.EC = D // 512                # 512-wide e-chunks of the output


def build(cfg: Cfg) -> bacc.Bacc:
    B, S, D, T, O = cfg.B, cfg.S, cfg.D, cfg.T, cfg.O
    H_PER, DC, TT, TM = cfg.H_PER, cfg.DC, cfg.TT, cfg.TM
    NJT, EC = cfg.NJT, cfg.EC
    scale = 1.0 / math.sqrt(HEAD_DIM)

    nc = bacc.Bacc(None, target_bir_lowering=False)

    # register -7.0 as a usable activation-bias constant (exp logit shift)
    EBIAS = -7.0
    _ebias_t = nc.alloc_sbuf_tensor("ebias_const", [128, 1], F32)
    nc.gpsimd.memset(_ebias_t.ap(), EBIAS)
    nc.const_aps.aps[(F32, EBIAS)] = _ebias_t.ap()

    # host-pre-tiled inputs (SBUF layout: partition-major)
    xtt = nc.dram_tensor("xtt", [TM, 128, DC, 256], F16, kind="ExternalInput")
    wqt = nc.dram_tensor("wqt", [128, DC, O], F16, kind="ExternalInput")
    wkt = nc.dram_tensor("wkt", [128, DC, O], F16, kind="ExternalInput")
    wvt = nc.dram_tensor("wvt", [128, DC, O], F16, kind="ExternalInput")
    wot = nc.dram_tensor("wot", [128, H_PER, D], F16, kind="ExternalInput")
    cost = nc.dram_tensor("cost", [128, TT, 64], F16, kind="ExternalInput")
    sint = nc.dram_tensor("sint", [128, TT, 64], F16, kind="ExternalInput")
    maskt = nc.dram_tensor("maskt", [128, 4, 512], F16, kind="ExternalInput")
    ident = nc.dram_tensor("ident", [128, 128], F16, kind="ExternalInput")
    onesm = nc.dram_tensor("onesm", [128, 128], F16, kind="ExternalInput")
    out = nc.dram_tensor("out", [T, D], F16, kind="ExternalOutput")

    # DRAM scratch for rope'd q/k in [token, O] layout
    q_s = nc.dram_tensor("q_s", [T, O], F16)
    k_s = nc.dram_tensor("k_s", [T, O], F16)

    with tile.TileContext(nc) as tc:
        with tc.tile_pool(name="const", bufs=1) as cpool, \
             tc.tile_pool(name="vres", bufs=1) as vpool:
            mask_sb = cpool.tile([128, 4, 512], F16, name="mask_sb")
            id_sb = cpool.tile([128, 128], F16, name="id_sb")
            ones_sb = cpool.tile([128, 128], F16, name="ones_sb")
            cos_sb = cpool.tile([128, TT, 64], F16, name="cos_sb")
            sin_sb = cpool.tile([128, TT, 64], F16, name="sin_sb")

            # V for all heads, [tok-in-tile, tile, O] — SBUF-resident
            v_all = vpool.tile([128, TT, O], F16, name="v_all")
            # ld tiles live outside the phase pools so attention's q/k loads
            # can prefetch during the projection tail
            ld_tiles = {w: [vpool.tile([128, 4, 128], F16, name=f"ld{w}{i}",
                                       tag=f"ld{w}{i}")
                            for i in range(8)] for w in ("q", "k")}

            # ============ Phase 1: Q,K,V projections + RoPE ============
            with tc.tile_pool(name="wqkv", bufs=1) as wpool, \
                 tc.tile_pool(name="p1sb", bufs=1) as sb, \
                 tc.tile_pool(name="p1ps", bufs=1, space="PSUM") as ps:
                wq_sb = wpool.tile([128, DC, O], F16, name="wq_sb")
                wk_sb = wpool.tile([128, DC, O], F16, name="wk_sb")
                wv_sb = wpool.tile([128, DC, O], F16, name="wv_sb")
                wtrip = ((wq_sb, wqt), (wk_sb, wkt), (wv_sb, wvt))

                def w_chunk(c0, cn):
                    for w_sb, wdram in wtrip:
                        nc.sync.dma_start(out=w_sb[:, c0:c0 + cn, :],
                                          in_=wdram[:, c0:c0 + cn, :])

                # single DMA queue is FIFO: order strictly by first need.
                # tm=0's x pieces are emitted here, interleaved with weights.
                x_sb0 = sb.tile([128, DC, 256], F16, name="x_sb", tag="x_sb", bufs=2)
                w_chunk(0, 2)
                nc.sync.dma_start(out=x_sb0[:, 0:8, :], in_=xtt[0][:, 0:8, :])
                w_chunk(2, 6)
                nc.sync.dma_start(out=x_sb0[:, 8:16, :], in_=xtt[0][:, 8:16, :])
                nc.sync.dma_start(out=mask_sb[:], in_=maskt[:])
                nc.sync.dma_start(out=id_sb[:], in_=ident[:])
                nc.sync.dma_start(out=ones_sb[:], in_=onesm[:])
                nc.sync.dma_start(out=cos_sb[:], in_=cost[:])
                nc.sync.dma_start(out=sin_sb[:], in_=sint[:])
                w_chunk(8, 8)
                nc.sync.dma_start(out=x_sb0[:, 16:32, :], in_=xtt[0][:, 16:32, :])
                w_chunk(16, 8)
                w_chunk(24, 8)

                rot_dmas = {"q": {}, "k": {}}  # tt_i -> rot write DMA inst

                def rope_unit(cp, tt_i, dst):
                    """cp: [128,512] f32 (copied psum); writes rot fp16 + DMA."""
                    rot = sb.tile([128, 512], F16, name="rot", tag="rot", bufs=3)
                    cosb = cos_sb[:, tt_i, :].unsqueeze(1).broadcast_to([128, H_PER, 64])
                    sinb = sin_sb[:, tt_i, :].unsqueeze(1).broadcast_to([128, H_PER, 64])
                    pv = cp[:].rearrange("p (h e k) -> p h e k", e=2, k=64)
                    rv = rot[:].rearrange("p (h e k) -> p h e k", e=2, k=64)
                    pe, po = pv[:, :, 0, :], pv[:, :, 1, :]
                    re, ro = rv[:, :, 0, :], rv[:, :, 1, :]
                    tmp = sb.tile([128, H_PER, 64], F32, name="tmp", tag="tmp", bufs=2)
                    tmp2 = sb.tile([128, H_PER, 64], F32, name="tmp2", tag="tmp2", bufs=2)
                    tmp3 = sb.tile([128, H_PER, 64], F32, name="tmp3", tag="tmp3", bufs=2)
                    nc.gpsimd.tensor_tensor(tmp[:], po, sinb, ALU.mult)
                    nc.gpsimd.tensor_tensor(tmp2[:], po, cosb, ALU.mult)
                    nc.gpsimd.tensor_tensor(tmp3[:], pe, sinb, ALU.mult)
                    nc.vector.tensor_tensor(re, pe, cosb, ALU.mult)
                    nc.vector.tensor_tensor(re, re, tmp[:], ALU.subtract)
                    nc.vector.tensor_tensor(ro, tmp2[:], tmp3[:], ALU.add)
                    dma = nc.sync.dma_start(
                        out=dst[tt_i * 128:(tt_i + 1) * 128, :], in_=rot[:])
                    rot_dmas["q" if dst is q_s else "k"][tt_i] = dma.ins

                for tm in range(TM):
                    if tm == 0:
                        x_sb = x_sb0
                    else:
                        x_sb = sb.tile([128, DC, 256], F16, name="x_sb",
                                       tag="x_sb", bufs=2)
                        for pc in range(2):
                            c0, c1 = DC * pc // 2, DC * (pc + 1) // 2
                            nc.sync.dma_start(out=x_sb[:, c0:c1, :],
                                              in_=xtt[tm][:, c0:c1, :])
                    pp = {}
                    for w in range(3):
                        for ts in range(2):
                            pp[w, ts] = ps.tile([128, 512], F32, name=f"pp{w}{ts}",
                                                tag=f"pp{w}{ts}", bufs=1)
                    for d in range(DC):
                        for ts in range(2):
                            lhs = x_sb[:, d, ts * 128:(ts + 1) * 128]
                            for w, w_sb in enumerate((wq_sb, wk_sb, wv_sb)):
                                nc.tensor.matmul(pp[w, ts][:], lhs, w_sb[:, d, :],
                                                 start=(d == 0), stop=(d == DC - 1))
                    # free psum banks first (in next-needed order), then rope math
                    cps = {}
                    for ts in range(2):
                        for w in range(2):
                            cps[w, ts] = sb.tile([128, 512], F32, name="cp",
                                                 tag=f"cp{w}{ts}", bufs=1)
                            nc.vector.tensor_copy(cps[w, ts][:], pp[w, ts][:])
                        nc.vector.tensor_copy(v_all[:, tm * 2 + ts, :], pp[2, ts][:])
                    for ts in range(2):
                        rope_unit(cps[0, ts], tm * 2 + ts, q_s)
                        rope_unit(cps[1, ts], tm * 2 + ts, k_s)

            # ================= Phase 2: attention per head ==============
            with tc.tile_pool(name="wo", bufs=1) as wopool, \
                 tc.tile_pool(name="otp", bufs=1) as otpool:
              wo_sb = wopool.tile([128, H_PER, D], F16, name="wo_sb")
              for hh in range(H_PER):
                  nc.sync.dma_start(out=wo_sb[:, hh, :], in_=wot[:, hh, :])
              ot_sb = [otpool.tile([128, T], F16, name=f"ot{h}") for h in range(H_PER)]

              with tc.tile_pool(name="atsb", bufs=1) as sb, \
                   tc.tile_pool(name="atps", bufs=1, space="PSUM") as ps:
                for h in range(H_PER):
                    qt_h = sb.tile([128, T], F16, name="qt_h", tag="qt_h", bufs=2)
                    kt_h = sb.tile([128, T], F16, name="kt_h", tag="kt_h", bufs=2)
                    for tp in range(TT // 4):
                        for which, src, dstt in (("q", q_s, qt_h), ("k", k_s, kt_h)):
                            ld = ld_tiles[which][tp % 8]
                            r0 = tp * 512
                            ld_dma = nc.sync.dma_start(
                                out=ld[:],
                                in_=src[r0:r0 + 512, h * 128:(h + 1) * 128]
                                .rearrange("(g p) e -> p g e", p=128))
                            for g in range(4):
                                add_dep_helper(ld_dma.ins,
                                               rot_dmas[which][tp * 4 + g],
                                               True, "q_s/k_s DRAM RAW")
                            for g in range(4):
                                ptr = ps.tile([128, 128], F16, name="ptr",
                                              tag="p_st", bufs=4)
                                nc.tensor.transpose(ptr[:], ld[:, g, :], id_sb[:])
                                nc.vector.tensor_copy(
                                    dstt[:, r0 + g * 128:r0 + (g + 1) * 128], ptr[:])

                    pt_st = sb.tile([128, NJT, 512], F16, name="pt_st",
                                    tag="pt_st", bufs=2)
                    for b in range(B):
                        for im in range(cfg.IMACS):
                            i0 = b * S + im * 512
                            njt = 4 * (im + 1)
                            # diag tiles first; k-th diag tile only covers
                            # columns >= 128k (rest is fully masked)
                            info = []
                            for k in range(4):
                                info.append((4 * im + k, 128 * k, True))
                            for jt in range(4 * im):
                                info.append((jt, 0, False))
                            p_ot = ps.tile([128, 512], F32, name="p_ot",
                                           tag="p_ot", bufs=2)
                            p_r = ps.tile([128, 512], F32, name="p_r",
                                          tag="p_r", bufs=2)

                            def consume(oi):
                                # mixed-extent accumulation group: the k=0
                                # diag tile (full width) opens it; later
                                # narrowed tiles accumulate sub-ranges. stop
                                # may land on a partial range (sim metadata
                                # only), hence skip_group_check.
                                jt, ist, _ = info[oi]
                                nc.tensor.matmul(
                                    p_r[:, ist:], ones_sb[:],
                                    pt_st[:, jt, ist:],
                                    start=(oi == 0), stop=(oi == njt - 1),
                                    skip_group_check=True)
                                nc.tensor.matmul(
                                    p_ot[:, ist:],
                                    v_all[:, b * NJT + jt, h * 128:(h + 1) * 128],
                                    pt_st[:, jt, ist:],
                                    start=(oi == 0), stop=(oi == njt - 1),
                                    skip_group_check=True)

                            LA = 2
                            for oi, (jt, ist, diag) in enumerate(info):
                                j0 = b * S + jt * 128
                                p_st = ps.tile([128, 512], F32, name="p_st",
                                               tag="p_st", bufs=4)
                                nc.tensor.matmul(p_st[:, ist:], kt_h[:, j0:j0 + 128],
                                                 qt_h[:, i0 + ist:i0 + 512],
                                                 start=True, stop=True)
                                # bias shifts all logits by a constant, which
                                # cancels in the softmax normalization; keeps
                                # exp() within fp16 range (logits up to ~18)
                                nc.scalar.activation(pt_st[:, jt, ist:],
                                                     p_st[:, ist:], AF.Exp,
                                                     bias=EBIAS, scale=scale)
                                if diag:
                                    # zero the causally-masked wedge (0/1 mask)
                                    nc.vector.tensor_tensor(
                                        pt_st[:, jt, ist:], pt_st[:, jt, ist:],
                                        mask_sb[:, jt - 4 * im, ist:], ALU.mult)
                                if oi >= LA:
                                    consume(oi - LA)
                            for oi in range(max(0, njt - LA), njt):
                                consume(oi)
                            r_sb = sb.tile([128, 512], F32, name="r_sb",
                                           tag="r_sb", bufs=2)
                            nc.vector.reciprocal_approx_fast(out=r_sb[:], in_=p_r[:])
                            nc.vector.tensor_tensor(ot_sb[h][:, i0:i0 + 512],
                                                    p_ot[:], r_sb[:], ALU.mult)

              # ============= Phase 3: output projection ===============
              with tc.tile_pool(name="p3sb", bufs=1) as sb3, \
                   tc.tile_pool(name="p3ps", bufs=1, space="PSUM") as ps3:
                  for tt_i in range(TT):
                      pf = [ps3.tile([128, 512], F32, name=f"pf{e}", tag=f"pf{e}")
                            for e in range(EC)]
                      for h in range(H_PER):
                          lhs = ot_sb[h][:, tt_i * 128:(tt_i + 1) * 128]
                          for e in range(EC):
                              nc.tensor.matmul(
                                  pf[e][:], lhs,
                                  wo_sb[:, h, e * 512:(e + 1) * 512],
                                  start=(h == 0), stop=(h == H_PER - 1))
                      f_sb = sb3.tile([128, D], F16, name="f_sb", tag="f_sb", bufs=3)
                      for e in range(EC):
                          nc.vector.tensor_copy(f_sb[:, e * 512:(e + 1) * 512],
                                                pf[e][:])
                          if e % 2 == 1:  # stream the output as halves complete
                              nc.sync.dma_start(
                                  out=out[tt_i * 128:(tt_i + 1) * 128,
                                          (e - 1) * 512:(e + 1) * 512],
                                  in_=f_sb[:, (e - 1) * 512:(e + 1) * 512])

    nc.compile()
    return nc


# host-side even/odd permutation of head dims (RoPE becomes half-split elementwise)
_PERM = np.concatenate([np.arange(0, HEAD_DIM, 2), np.arange(1, HEAD_DIM, 2)])


def host_inputs(cfg: Cfg, x, wq, wk, wv, wo, freqs_cos, freqs_sin):
    """Build the 8 per-core input maps from full inputs (numpy f32)."""
    B, S, D, T, O, H_PER = cfg.B, cfg.S, cfg.D, cfg.T, cfg.O, cfg.H_PER
    TM, DC, TT = cfg.TM, cfg.DC, cfg.TT

    x16 = np.asarray(x, np.float32).reshape(T, D).astype(np.float16)
    # [TM, 128, DC, 256]: xtt[tm, p, dc, j] = x[tm*256 + j, dc*128 + p]
    xtt = np.ascontiguousarray(
        x16.reshape(TM, 256, DC, 128).transpose(0, 3, 2, 1))

    def tile_w(w_cols):  # [D, O] -> [128, DC, O]
        return np.ascontiguousarray(
            w_cols.astype(np.float16).reshape(DC, 128, O).transpose(1, 0, 2))

    cos_full = np.tile(np.asarray(freqs_cos, np.float32), (B, 1)).astype(np.float16)
    sin_full = np.tile(np.asarray(freqs_sin, np.float32), (B, 1)).astype(np.float16)
    cost = np.ascontiguousarray(cos_full.reshape(TT, 128, 64).transpose(1, 0, 2))
    sint = np.ascontiguousarray(sin_full.reshape(TT, 128, 64).transpose(1, 0, 2))

    # maskt[p, k, i] = 1 if 128k + p <= i else 0 (multiplicative causal mask)
    j_idx = (np.arange(4)[None, :, None] * 128 + np.arange(128)[:, None, None])
    i_idx = np.arange(512)[None, None, :]
    maskt = np.where(j_idx <= i_idx, 1.0, 0.0).astype(np.float16)

    ident = np.eye(128, dtype=np.float16)
    onesm = np.ones((128, 128), dtype=np.float16)

    wq = np.asarray(wq, np.float32)
    wk = np.asarray(wk, np.float32)
    wv = np.asarray(wv, np.float32)
    wo = np.asarray(wo, np.float32)

    in_maps = []
    for c in range(cfg.n_cores):
        rows = []
        for hh in range(H_PER):
            base = (c * H_PER + hh) * HEAD_DIM
            rows.append(base + _PERM)
        prows = np.concatenate(rows)                     # permuted rows for q/k
        nrows = np.arange(c * O, (c + 1) * O)            # natural rows for v
        wot_t = np.ascontiguousarray(
            wo[:, nrows].T.astype(np.float16).reshape(H_PER, 128, D)
            .transpose(1, 0, 2))
        in_maps.append({
            "xtt": xtt,
            "wqt": tile_w(wq[prows].T),
            "wkt": tile_w(wk[prows].T),
            "wvt": tile_w(wv[nrows].T),
            "wot": wot_t,
            "cost": cost, "sint": sint, "maskt": maskt,
            "ident": ident, "onesm": onesm,
        })
    return in_maps


_CACHE = {}


def kernel(x, wq, wk, wv, wo, freqs_cos, freqs_sin, mask=None, start_pos=0):
    cfg = Cfg()
    in_maps = host_inputs(cfg, x, wq, wk, wv, wo, freqs_cos, freqs_sin)
    if "nc" not in _CACHE:
        _CACHE["nc"] = build(cfg)
    from concourse.bass_utils import run_bass_kernel_spmd
    res = run_bass_kernel_spmd(_CACHE["nc"], in_maps, core_ids=list(range(cfg.n_cores)))
    acc = np.zeros((cfg.T, cfg.D), dtype=np.float32)
    for c in range(cfg.n_cores):
        acc += res.results[c]["out"].astype(np.float32)
    return acc.reshape(cfg.B, cfg.S, cfg.D)
